# revision 1
# baseline (speedup 1.0000x reference)
"""Trainium2 Bass kernel for a channel-attention block.

Per batch b (one NeuronCore each, 8 total):
    v      = x[b].reshape(C, H*W)                    # [256, 16384]
    energy = v @ v.T                                 # [256, 256]
    w      = softmax(max(energy, -1) - energy, -1)   # == softmax(-energy)
    y      = alpha * (w @ v) + x[b]

Layout / strategy (per core):
  - v stays resident in SBUF as [128, 2, 16384] f32r (c = h*128 + p).
  - energy needs s on partitions, so each 128-wide s-tile of v is
    transposed on the PE, then fed to two float32r matmuls (FP22
    truncation, full bf16-rate at N>=256) accumulating [128, 256] PSUM
    tiles over all 128 s-tiles.  The transpose+copy for k+1 is emitted
    one iteration ahead of k's matmuls (software pipeline) and the
    PSUM->SBUF copies alternate ScalarE/VectorE.
  - Stable softmax via one reduce-min + one fused ScalarE
    exp(-energy + rowmin) with accumulated row-sum, then reciprocal
    multiply.  (softmax(max-e) == exp(rowmin-e)/sum.)
  - w is PE-transposed to wT; second matmul contracts over channels with
    v in natural layout; alpha*psum + x fused on VectorE; 2 MB staged
    output DMAs.  The first and last output chunks are computed
    jj-major and drained in 512 KB quarters so the output stream starts
    right after softmax and the final (kernel-gating) DMA is small.
    PSUM pools are scoped per phase: B/C get 6 transpose banks, D gets
    4 output banks.
"""

from contextlib import ExitStack

import numpy as np

import concourse.bass as bass
import concourse.mybir as mybir
import concourse.tile as tile
from concourse import bacc
from concourse.bass_utils import run_bass_kernel_spmd
from concourse.masks import make_identity

B, C, HH, WW = 8, 256, 128, 128
HW = HH * WW            # 16384
P = 128
H = C // P              # 2 channel chunks
KT = HW // P            # 128 contraction tiles for energy
S_CHUNK = 2048          # columns per streaming DMA chunk (2 MB)
N_CHUNKS = HW // S_CHUNK
S_TILE = 512            # second-matmul moving free dim (1 PSUM bank)
J_PER_CHUNK = S_CHUNK // S_TILE

F32 = mybir.dt.float32
F32R = mybir.dt.float32r


def emit(nc, tc, alpha, ident_r, v_sb, x_v, y_v):
    """One full per-core pass (phases A-D). Pools are scoped inside."""
    # ---- Phase A: stream x into SBUF.  The first chunk is split into
    # quarters so the PE can start transposing after ~512 KB instead of 2 MB.
    for q in range(4):
        sl = slice(q * (S_CHUNK // 4), (q + 1) * (S_CHUNK // 4))
        nc.sync.dma_start(out=v_sb[:, :, sl], in_=x_v[:, :, sl])
    for ck in range(1, N_CHUNKS):
        sl = slice(ck * S_CHUNK, (ck + 1) * S_CHUNK)
        nc.sync.dma_start(out=v_sb[:, :, sl], in_=x_v[:, :, sl])

    with ExitStack() as wctx:
        w_pool = wctx.enter_context(tc.tile_pool(name="w", bufs=1))
        w_sb = [w_pool.tile([P, C], F32R, name=f"w{h}") for h in range(H)]
        wt_sb = [w_pool.tile([P, C], F32R, name=f"wt{g}") for g in range(H)]

        with ExitStack() as bctx:
            vt_pool = bctx.enter_context(tc.tile_pool(name="vt", bufs=8))
            stats = bctx.enter_context(tc.tile_pool(name="stats", bufs=1))
            psum_e = bctx.enter_context(
                tc.tile_pool(name="psum_e", bufs=1, space="PSUM"))
            psum_t = bctx.enter_context(
                tc.tile_pool(name="psum_t", bufs=6, space="PSUM"))

            # ---- Phase B: energy = v @ v.T (two PSUM banks), with the
            # transpose+copy pipelined one k ahead of the matmuls.
            e_ps = [psum_e.tile([P, C], F32, name=f"energy{h}")[:]
                    for h in range(H)]

            def make_vt(k):
                ksl = slice(k * P, (k + 1) * P)
                vt = vt_pool.tile([P, C], F32R, name="vt")
                tp = psum_t.tile([P, C], F32R, name="tp")
                for h in range(H):
                    nc.tensor.transpose(
                        tp[:, h * P:(h + 1) * P], v_sb[:, h, ksl], ident_r[:]
                    )
                if k % 2 == 0:
                    nc.scalar.copy(vt[:], tp[:])
                else:
                    nc.vector.tensor_copy(vt[:], tp[:])
                return vt

            # Two k-tiles per pipeline step: their 4 matmuls run
            # back-to-back so the self-loading weight fetch of each next
            # matmul pulls ahead of the running one (PE reorder window).
            vts = [make_vt(0), make_vt(1)]
            for k0 in range(0, KT, 2):
                for kn in (k0 + 2, k0 + 3):
                    if kn < KT:
                        vts.append(make_vt(kn))
                for k in (k0, k0 + 1):
                    vt_r = vts.pop(0)[:]
                    for h in range(H):
                        nc.tensor.matmul(
                            e_ps[h],
                            lhsT=vt_r[:, h * P:(h + 1) * P],
                            rhs=vt_r,
                            start=(k == 0),
                            stop=(k == KT - 1),
                        )

            # ---- Phase C: row softmax of (max - e) == exp(min - e)/sum.
            for h in range(H):
                mn = stats.tile([P, 1], F32, name=f"mn{h}")
                sm = stats.tile([P, 1], F32, name=f"sm{h}")
                rc = stats.tile([P, 1], F32, name=f"rc{h}")
                nc.vector.tensor_reduce(
                    mn[:], e_ps[h], axis=mybir.AxisListType.X,
                    op=mybir.AluOpType.min
                )
                nc.scalar.activation(
                    w_sb[h][:], e_ps[h], mybir.ActivationFunctionType.Exp,
                    bias=mn[:], scale=-1.0, accum_out=sm[:],
                )
                nc.vector.reciprocal(rc[:], sm[:])
                nc.vector.tensor_scalar_mul(w_sb[h][:], w_sb[h][:], rc[:])
            # wT[g][p, h*128+q] = w[h][q, g*128+p] for the second matmul.
            for g in range(H):
                for h in range(H):
                    tp2 = psum_t.tile([P, C], F32R, name="tp2", tag="tp")
                    nc.tensor.transpose(
                        tp2[:, 0:P], w_sb[h][:, g * P:(g + 1) * P], ident_r[:]
                    )
                    nc.vector.tensor_copy(
                        wt_sb[g][:, h * P:(h + 1) * P], tp2[:, 0:P])

        # ---- Phase D: y = alpha*(w @ v) + v, streamed out in 2 MB chunks.
        # First and last chunks run jj-major and drain in 512 KB quarters
        # (earlier stream start / small kernel-gating final DMA).
        with ExitStack() as dctx:
            out_pool = dctx.enter_context(tc.tile_pool(name="out", bufs=3))
            psum_y = dctx.enter_context(
                tc.tile_pool(name="psum_y", bufs=4, space="PSUM"))

            def tile_mm_stt(ost, ck, m, jj):
                j0 = ck * S_CHUNK + jj * S_TILE
                jsl = slice(j0, j0 + S_TILE)
                yp = psum_y.tile([P, S_TILE], F32, name="yp")
                for g in range(H):
                    nc.tensor.matmul(
                        yp[:],
                        lhsT=wt_sb[g][:][:, m * P:(m + 1) * P],
                        rhs=v_sb[:][:, g, jsl],
                        start=(g == 0),
                        stop=(g == H - 1),
                    )
                nc.vector.scalar_tensor_tensor(
                    out=ost[:, m, jj * S_TILE:(jj + 1) * S_TILE],
                    in0=yp[:],
                    scalar=alpha,
                    in1=v_sb[:, m, jsl].bitcast(F32),
                    op0=mybir.AluOpType.mult,
                    op1=mybir.AluOpType.add,
                )

            for ck in range(N_CHUNKS):
                ost = out_pool.tile([P, H, S_CHUNK], F32, name="ost")
                if ck == 0 or ck == N_CHUNKS - 1:
                    # jj-major so each 512-col quarter completes (both m
                    # halves) before the next starts; DMA per quarter.
                    for jj in range(J_PER_CHUNK):
                        for m in range(H):
                            tile_mm_stt(ost, ck, m, jj)
                        osl = slice(ck * S_CHUNK + jj * S_TILE,
                                    ck * S_CHUNK + (jj + 1) * S_TILE)
                        nc.sync.dma_start(
                            out=y_v[:, :, osl],
                            in_=ost[:, :, jj * S_TILE:(jj + 1) * S_TILE])
                else:
                    for m in range(H):
                        for jj in range(J_PER_CHUNK):
                            tile_mm_stt(ost, ck, m, jj)
                    osl = slice(ck * S_CHUNK, (ck + 1) * S_CHUNK)
                    nc.sync.dma_start(out=y_v[:, :, osl], in_=ost[:])


def _build(alpha: float) -> bass.Bass:
    # Bacc (not plain Bass): its compile() legalizes semaphore waits into
    # EventSemaphore instructions — hardware allows only 1 wait per
    # instruction and Tile freely emits more.
    nc = bacc.Bacc("TRN2", target_bir_lowering=False)
    # x is declared float32r (same 32-bit layout as fp32 at rest) so the DMA,
    # the PE transposes, and both matmuls form a consistent f32r chain for
    # the BIR verifier; the PE truncates to FP22 on read either way.
    x = nc.dram_tensor("x", [C, HW], F32R, kind="ExternalInput")
    y = nc.dram_tensor("y", [C, HW], F32, kind="ExternalOutput")
    x_v = x.rearrange("(h p) s -> p h s", p=P)
    y_v = y.rearrange("(h p) s -> p h s", p=P)

    with tile.TileContext(nc) as tc, ExitStack() as ctx:
        singles = ctx.enter_context(tc.tile_pool(name="singles", bufs=1))
        ident = singles.tile([P, P], F32, name="ident")
        make_identity(nc, ident)
        ident_r = singles.tile([P, P], F32R, name="ident_r")
        nc.vector.tensor_copy(ident_r[:], ident[:])
        # Whole v resident: 128 KB per partition.
        v_sb = singles.tile([P, H, HW], F32R, name="v_sb")
        emit(nc, tc, alpha, ident_r, v_sb, x_v, y_v)
    nc.compile()
    return nc


def kernel(x: np.ndarray, alpha: np.ndarray, **_kw) -> np.ndarray:
    assert x.shape == (B, C, HH, WW) and x.dtype == np.float32
    xs = np.ascontiguousarray(x.reshape(B, C, HW)).astype(np.float32, copy=False)
    nc = _build(float(np.asarray(alpha).reshape(-1)[0]))
    in_maps = [{"x": xs[b]} for b in range(B)]
    res = run_bass_kernel_spmd(nc, in_maps, core_ids=list(range(B)))
    out = np.stack([np.asarray(r["y"]) for r in res.results])
    return out.reshape(B, C, HH, WW).astype(np.float32, copy=False)

